# revision 8
# baseline (speedup 1.0000x reference)
"""Trainium2 Bass kernel for BatchMatchedMSELoss.

loss = mean_i min_j mean_d (input[i,d] - target[j,d])^2

Decomposition:
  mse[i,j]  = (||x_i||^2 + ||t_j||^2 - 2<x_i, t_j>) / D
  min_j mse = (||x_i||^2 + min_j(||t_j||^2 - 2<x_i,t_j>)) / D

Device (per core c, rows i in [c*1024, (c+1)*1024)), j on PSUM partitions:
  psum[jp, i] = 2<x_i, t_j>            (bf16 matmul, K=256, f32 accum)
  m'[jp, i]   = psum - tgsqc_j         (per-partition bias)
  rowmax'[i]  = max over all j of m'   -> rowmin = -rowmax'
The bias+max-accumulate runs as one fused DVE scalar_tensor_tensor per
j-tile; a fraction of j-tiles instead do bias on the (otherwise idle)
Activation engine + a cheap fp16 tensor_tensor max on DVE, balancing the
two engines.  A log2 partition-halving max tree finishes the reduction.
Host adds ||x_i||^2 + C (tgsq centering constant), /D, and means.

Sharding: data-parallel over input rows, 8 cores, target replicated.
No collectives; per-core [1, 1024] row-min vectors are gathered on host.
"""

import os
import sys

sys.path.insert(0, "/opt/trn_rl_repo")

import numpy as np
import ml_dtypes

B = 8192
D = 256
NCORES = 8
MS = B // NCORES  # 1024 rows (i) per core
P = 128
KC = D // P  # 2 contraction chunks
JT = B // P  # 64 j-tiles
NT = 512  # matmul free-dim tile (one PSUM bank)
HS = MS // NT  # 2 i-subtiles per psum tile
TGT_CHUNK = 2048  # dma chunk width for target
# j-tiles whose bias runs on DVE (fused STT); the rest use ACT + TT-max.
# Pattern period 3 -> ~1/3 on DVE, 2/3 on ACT.
DVE_EVERY = 3

_CACHE = {}


def _build_nc():
    from contextlib import ExitStack

    import concourse.bacc as bacc
    import concourse.tile as tile
    import concourse.mybir as mybir

    bf16 = mybir.dt.bfloat16
    fp16 = mybir.dt.float16
    f32 = mybir.dt.float32

    nc = bacc.Bacc("TRN2", target_bir_lowering=False, debug=False)

    # tgtT: target.T (bf16), inT: (2*input_shard).T (bf16)
    tgtT_d = nc.dram_tensor("tgtT", [D, B], bf16, kind="ExternalInput").ap()
    inT_d = nc.dram_tensor("inT", [D, MS], bf16, kind="ExternalInput").ap()
    # negtg[p, jt] = -(||t_j||^2 - C) for j = jt*128 + p
    negtg_d = nc.dram_tensor("negtg", [P, JT], f32, kind="ExternalInput").ap()
    # rowmax' partials: host finishes the 128-way partition max
    out_d = nc.dram_tensor("rowmax", [P, MS], fp16, kind="ExternalOutput").ap()

    with tile.TileContext(nc) as tc, ExitStack() as ctx:
        persist = ctx.enter_context(tc.tile_pool(name="persist", bufs=1))
        psum_pool = ctx.enter_context(tc.tile_pool(name="psum", bufs=4, space="PSUM"))
        m_pool = ctx.enter_context(tc.tile_pool(name="m", bufs=4))

        # --- persistent SBUF buffers ---
        tgtT_sb = [
            [
                persist.tile([P, TGT_CHUNK], bf16, name=f"tgtT{k}_{c}", tag=f"tgtT{k}_{c}")
                for c in range(B // TGT_CHUNK)
            ]
            for k in range(KC)
        ]
        inT_sb = [persist.tile([P, MS], bf16, name=f"inT{k}", tag=f"inT{k}") for k in range(KC)]
        negtg_sb = persist.tile([P, JT], f32, name="negtg_sb", tag="negtg_sb")
        acc = persist.tile([P, MS], fp16, name="acc", tag="acc")

        # --- loads ---
        for k in range(KC):
            nc.sync.dma_start(out=inT_sb[k][:], in_=inT_d[k * P : (k + 1) * P, :])
        nc.sync.dma_start(out=negtg_sb[:], in_=negtg_d[:, :])
        for c in range(B // TGT_CHUNK):
            for k in range(KC):
                nc.sync.dma_start(
                    out=tgtT_sb[k][c][:],
                    in_=tgtT_d[k * P : (k + 1) * P, c * TGT_CHUNK : (c + 1) * TGT_CHUNK],
                )

        # --- main loop over j-tiles ---
        for jt in range(JT):
            c, off = jt // (TGT_CHUNK // P), (jt % (TGT_CHUNK // P)) * P
            ps = psum_pool.tile([P, MS], f32)
            for k in range(KC):
                for h in range(HS):
                    nc.tensor.matmul(
                        ps[:, h * NT : (h + 1) * NT],
                        tgtT_sb[k][c][:, off : off + P],
                        inT_sb[k][:, h * NT : (h + 1) * NT],
                        start=(k == 0),
                        stop=(k == KC - 1),
                    )
            bias_col = negtg_sb[:, jt : jt + 1]
            if jt == 0:
                # initialize acc = ps + bias (fp16) via ACT
                nc.scalar.activation(
                    out=acc[:],
                    in_=ps[:],
                    func=mybir.ActivationFunctionType.Identity,
                    bias=bias_col,
                    scale=1.0,
                )
            elif jt % DVE_EVERY == 0:
                # fused bias + max-accumulate on DVE
                nc.vector.scalar_tensor_tensor(
                    out=acc[:],
                    in0=ps[:],
                    scalar=bias_col,
                    in1=acc[:],
                    op0=mybir.AluOpType.add,
                    op1=mybir.AluOpType.max,
                )
            else:
                # bias on ACT, cheap fp16 max on DVE
                m_t = m_pool.tile([P, MS], fp16)
                nc.scalar.activation(
                    out=m_t[:],
                    in_=ps[:],
                    func=mybir.ActivationFunctionType.Identity,
                    bias=bias_col,
                    scale=1.0,
                )
                nc.vector.tensor_tensor(
                    acc[:], acc[:], m_t[:], op=mybir.AluOpType.max
                )

        nc.sync.dma_start(out=out_d[:, :], in_=acc[:])

    nc.compile()
    return nc


def _get_nc():
    if "nc" not in _CACHE:
        _CACHE["nc"] = _build_nc()
    return _CACHE["nc"]


LAST_RESULTS = None  # BassKernelResults of the most recent run (for test harness)


def _install_ntff_hook_shim():
    """The image's antenv lacks axon_hooks; register an equivalent module so
    run_bass_kernel_spmd(trace=True) can capture NTFF profiles via the axon
    ctypes path.  Harmless when tracing is off."""
    import types

    try:
        import antenv.axon_hooks  # noqa: F401

        return
    except ImportError:
        pass
    hook = None
    try:
        from trn_agent_boot.trn_boot import _ntff_profile_via_ctypes

        hook = _ntff_profile_via_ctypes("/opt/axon/libaxon_pjrt.so")
    except Exception:
        pass
    import antenv

    mod = types.ModuleType("antenv.axon_hooks")
    mod.get_axon_ntff_profile_hook = lambda: hook
    mod.set_axon_ntff_profile_hook = lambda h: None
    sys.modules["antenv.axon_hooks"] = mod
    antenv.axon_hooks = mod


def kernel(input, target):
    global LAST_RESULTS
    from concourse.bass_utils import run_bass_kernel_spmd

    _install_ntff_hook_shim()

    nc = _get_nc()

    inp = np.asarray(input, dtype=np.float32)
    tgt = np.asarray(target, dtype=np.float32)
    assert inp.shape == (B, D) and tgt.shape == (B, D)

    tgtT_np = np.ascontiguousarray(tgt.T).astype(ml_dtypes.bfloat16)  # [D, B]
    inT_full = np.ascontiguousarray((2.0 * inp).T).astype(ml_dtypes.bfloat16)  # [D, B]
    tgsq = np.sum(tgt.astype(np.float64) ** 2, axis=1)
    C = float(tgsq.mean())
    # negtg[p, jt] = -(tgsq[jt*128+p] - C)
    negtg = np.ascontiguousarray(
        -(tgsq - C).astype(np.float32).reshape(JT, P).T
    )

    in_maps = [
        {
            "tgtT": tgtT_np,
            "inT": np.ascontiguousarray(inT_full[:, c * MS : (c + 1) * MS]),
            "negtg": negtg,
        }
        for c in range(NCORES)
    ]

    trace = bool(int(os.environ.get("KERNEL_TRACE", "0")))
    res = run_bass_kernel_spmd(nc, in_maps, core_ids=list(range(NCORES)), trace=trace)
    LAST_RESULTS = res

    rowmin = np.concatenate(
        [-res.results[c]["rowmax"].astype(np.float32).max(axis=0) for c in range(NCORES)]
    )
    in_sq = np.sum(inp.astype(np.float64) ** 2, axis=1)
    loss = np.mean((in_sq + C + rowmin.astype(np.float64)) / float(D))
    return np.asarray(loss, dtype=np.float32)
